# revision 5
# baseline (speedup 1.0000x reference)
"""CRF negative log-likelihood on 8 Trainium2 NeuronCores.

Problem: B=128, T=2048, K=96 linear-chain CRF loss (log-partition via the
forward algorithm minus the joint path score), mask is all-ones.

Strategy (v2)
-------------
Batch dim B is sharded 16 sequences per core (data parallel); the [K,K]
transition matrix is replicated.  The host splits the time axis in two:
lgf = logits[:, 0:1024] and lgb = logits[:, 2047:1023:-1] (back half
time-reversed), both bf16-cast and K-padded to 128.  With r the local
index in either half, the forward chain (p_r = xf_r * E^T p_{r-1}) and
the backward chain (w_r = xb_r * E w_{r-1}) share the same r sequencing
and meet in the middle: Z = p_1023^T E w_1023.

Per core, per chain step: TWO tiny PE matmuls ([96x96] @ [96,16], fwd
and bwd) into one PSUM tile [96,32], then ONE merged DVE multiply with a
paired xhat tile (xf_r | xb_r).  The pairing halves the DVE instruction
count vs per-chain multiplies, amortizing the fixed ~120-cycle PSUM
access per DVE op.  xhat pair tiles are produced with a fused path: a
DMA xbar-transpose loads logits^T ([128k,128t] bf16) and one ACT exp
writes straight into the strided pair-tile slot -- no PE transposes.

The joint score runs off the critical path: per (b, 128-step tile)
one-hot label tiles (DVE compare vs iota) give the emission score via a
fused multiply-reduce, and the transition score via PE pair-count
matmuls PSUM-accumulated over each sequence (16 matmuls -> ONE fused
<count, transitions> reduce per sequence).  The time-reversed back half
counts pairs in the same (i->j) orientation because the host supplies
labnext = label successors in (half, r) layout; the single spurious
pair per sequence is subtracted with a tiny one-hot/matmul batch.

Each core returns a small vector of partial sums; the host only
assembles the final scalar: loss = -sum_b (score_b - logZ_b).
"""
import sys

sys.path.insert(0, "/opt/trn_rl_repo")

import numpy as np
import ml_dtypes

import concourse.bacc as bacc
import concourse.bass as bass
import concourse.mybir as mybir
from concourse.bass_utils import run_bass_kernel_spmd
from concourse.tile import TileContext

B, T, K = 128, 2048, 96
KP = 128                   # K padded for the xbar transpose
N_CORES = 8
BL = B // N_CORES          # 16 sequences per core
C0 = 5.06                  # per-step scale offset, ~E[log growth]
CHUNK = 128                # time-steps per tile
HALF = T // 2              # 1024
NCH = HALF // CHUNK        # 8 chunks per half
NTILE = 2 * NCH            # 16 (ch2 = c*2 + h indexing over (c, h))
NQ = NTILE * BL            # 256 quanta
F32 = mybir.dt.float32
BF16 = mybir.dt.bfloat16
I32 = mybir.dt.int32
EXP = mybir.ActivationFunctionType.Exp
MULT = mybir.AluOpType.mult
EQ = mybir.AluOpType.is_equal

# stackA columns: [0:16] zpre, [16:32] start, [32:48] end, [48:64] trans,
# [64:80] negative pair corrections.  stackB: [128, 256] emission partials.
AW = 5 * BL
OUT_W = AW + NQ


def build_program():
    nsteps = HALF - 1                  # 1023 chain steps (r = 1..1023)

    nc = bacc.Bacc(None, target_bir_lowering=False)
    lgf_in = nc.declare_dram_parameter("lgf", [BL, HALF, KP], BF16, isOutput=False)
    lgb_in = nc.declare_dram_parameter("lgb", [BL, HALF, KP], BF16, isOutput=False)
    labc_in = nc.declare_dram_parameter("labcur", [BL, T], F32, isOutput=False)
    labn_in = nc.declare_dram_parameter("labnext", [BL, T], F32, isOutput=False)
    tr_in = nc.declare_dram_parameter("transitions", [K, K], F32, isOutput=False)
    st_in = nc.declare_dram_parameter("start_t", [K, 1], F32, isOutput=False)
    en_in = nc.declare_dram_parameter("end_t", [K, 1], F32, isOutput=False)
    # rows: labels[:,0], labels[:,T-1] (f32) for start/end/correction scores
    le_in = nc.declare_dram_parameter("lab_edge", [2, BL], F32, isOutput=False)
    y_out = nc.declare_dram_parameter("y", [1, OUT_W], F32, isOutput=True)

    # (c, h) order for ch2 so XP[c] completes as early as possible
    CH2 = [(c, h) for c in range(NCH) for h in range(2)]

    with TileContext(nc) as tc:
        with (
            tc.tile_pool(name="const", bufs=1) as cpool,
            tc.tile_pool(name="xp", bufs=1) as xpool,
            tc.tile_pool(name="ohc", bufs=1) as ohpool,
            tc.tile_pool(name="em", bufs=1) as empool,
            tc.tile_pool(name="emt", bufs=1) as emtpool,
            tc.tile_pool(name="state", bufs=4) as stpool,
            tc.tile_pool(name="scr", bufs=4) as scrpool,
            tc.tile_pool(name="stacks", bufs=1) as kpool,
            tc.tile_pool(name="ps", bufs=3, space="PSUM") as pspool,
            tc.tile_pool(name="cps", bufs=2, space="PSUM") as cpspool,
            tc.tile_pool(name="pu", bufs=1, space="PSUM") as pupool,
            tc.tile_pool(name="po", bufs=1, space="PSUM") as popool,
        ):
            # ---- constants -------------------------------------------------
            tr_f = cpool.tile([K, K], F32, tag="tr_f")
            trT_f = cpool.tile([K, K], F32, tag="trT_f")
            nc.sync.dma_start(out=tr_f[:], in_=tr_in[:])
            nc.sync.dma_start(out=trT_f[:], in_=tr_in[:].rearrange("i j -> j i"))
            e_sb = cpool.tile([K, K], BF16, tag="e_sb")
            eT_sb = cpool.tile([K, K], BF16, tag="eT_sb")
            nc.scalar.activation(e_sb[:], tr_f[:], EXP)
            nc.scalar.activation(eT_sb[:], trT_f[:], EXP)

            st_col = cpool.tile([K, 1], F32, tag="st_col")
            en_col = cpool.tile([K, 1], F32, tag="en_col")
            nc.sync.dma_start(out=st_col[:], in_=st_in[:])
            nc.sync.dma_start(out=en_col[:], in_=en_in[:])
            een_col = cpool.tile([K, 1], F32, tag="een_col")
            nc.scalar.activation(een_col[:], en_col[:], EXP)
            labs0 = cpool.tile([K, BL], F32, tag="labs0")
            labs1 = cpool.tile([K, BL], F32, tag="labs1")
            nc.sync.dma_start(out=labs0[:], in_=le_in[0:1, :].to_broadcast([K, BL]))
            nc.sync.dma_start(out=labs1[:], in_=le_in[1:2, :].to_broadcast([K, BL]))
            iotac_i = cpool.tile([K, 1], I32, tag="iotac_i")
            nc.gpsimd.iota(iotac_i[:], pattern=[[1, 1]], base=0, channel_multiplier=1)
            iotac = cpool.tile([K, 1], F32, tag="iotac")
            nc.vector.tensor_copy(iotac[:], iotac_i[:])

            negc0k = cpool.tile([K, 1], F32, tag="negc0k")
            nc.vector.memset(negc0k[:], -C0)
            posc0 = cpool.tile([K, 1], F32, tag="posc0")
            nc.vector.memset(posc0[:], C0)
            stc0 = cpool.tile([K, 1], F32, tag="stc0")
            nc.scalar.activation(stc0[:], st_col[:], EXP, bias=posc0[:])
            iota_i = cpool.tile([CHUNK, K], I32, tag="iota_i")
            nc.gpsimd.iota(iota_i[:], pattern=[[1, K]], base=0, channel_multiplier=0)
            iota = cpool.tile([CHUNK, K], BF16, tag="iota")
            nc.vector.tensor_copy(iota[:], iota_i[:])
            ones96 = cpool.tile([K, 1], F32, tag="ones96")
            ones128 = cpool.tile([CHUNK, 1], F32, tag="ones128")
            nc.vector.memset(ones96[:], 1.0)
            nc.vector.memset(ones128[:], 1.0)

            lab_sb = []
            labn_sb = []
            for b in range(BL):
                lt = cpool.tile([CHUNK, NTILE], F32, tag=f"lab{b}")
                nc.sync.dma_start(
                    out=lt[:],
                    in_=labc_in[b : b + 1, :].rearrange("o (c t) -> (o t) c", t=CHUNK),
                )
                lab_sb.append(lt)
                ln = cpool.tile([CHUNK, NTILE], F32, tag=f"labn{b}")
                nc.sync.dma_start(
                    out=ln[:],
                    in_=labn_in[b : b + 1, :].rearrange("o (c t) -> (o t) c", t=CHUNK),
                )
                labn_sb.append(ln)

            stackA = kpool.tile([K, AW], F32, tag="stackA")
            stackB = kpool.tile([CHUNK, NQ], F32, tag="stackB")
            outstage = kpool.tile([1, OUT_W], F32, tag="outstage")
            nc.vector.memset(stackA[:], 0.0)

            # xhat pair tiles: XP[c][:, ti, 0:16] = xf at r=c*128+ti (16 seqs),
            # [:, ti, 16:32] = xb at the same r.
            XP = [
                xpool.tile([K, CHUNK, 2 * BL], BF16, tag=f"xp{c}", name=f"xp{c}")
                for c in range(NCH)
            ]

            em_tiles = {}
            ohc_tiles = {}

            def xh_task(q):
                # DMA xbar-transpose load + fused exp into the pair slot.
                ch2, b = divmod(q, BL)
                c, h = CH2[ch2]
                src = lgf_in if h == 0 else lgb_in
                emt = emtpool.tile(
                    [KP, CHUNK], BF16, tag=f"emt{q % 12}", name=f"emt{q}"
                )
                nc.sync.dma_start_transpose(
                    emt[:], src[b, c * CHUNK : (c + 1) * CHUNK, :]
                )
                em = empool.tile(
                    [CHUNK, KP], BF16, tag=f"em{q % 96}", name=f"em{q}"
                )
                nc.gpsimd.dma_start(
                    out=em[:], in_=src[b, c * CHUNK : (c + 1) * CHUNK, :]
                )
                em_tiles[q] = em
                off = b + BL * h
                nc.scalar.activation(
                    XP[c][:, :, off], emt[0:K, :], EXP, bias=negc0k[:]
                )

            def ohc_task(o):
                ch2, b = divmod(o, BL)
                oh = ohpool.tile([CHUNK, K], BF16, tag=f"ohc{o}", name=f"ohc{o}")
                nc.vector.tensor_scalar(
                    oh[:], iota[:], lab_sb[b][:, ch2 : ch2 + 1], None, op0=EQ
                )
                ohc_tiles[o] = oh

            def emit_task(s):
                ch2, b = divmod(s, BL)
                em = em_tiles.pop(s)
                scr = scrpool.tile([CHUNK, K], BF16, tag="scr")
                nc.vector.scalar_tensor_tensor(
                    out=scr[:],
                    in0=ohc_tiles[s][:],
                    scalar=1.0,
                    in1=em[:, 0:K],
                    op0=MULT,
                    op1=MULT,
                    accum_out=stackB[:, s : s + 1],
                )

            cps_cur = [None]

            def count_task(n):
                b, ch2 = divmod(n, NTILE)
                o = ch2 * BL + b
                oh = ohc_tiles[o]
                ohn = scrpool.tile([CHUNK, K], BF16, tag=f"ohn{n % 4}")
                nc.vector.tensor_scalar(
                    ohn[:], iota[:], labn_sb[b][:, ch2 : ch2 + 1], None, op0=EQ
                )
                if ch2 == 0:
                    cps_cur[0] = cpspool.tile([K, K], F32, tag="cps", name="cps")
                cps = cps_cur[0]
                nc.tensor.matmul(
                    cps[:], oh[:], ohn[:], start=(ch2 == 0), stop=(ch2 == NTILE - 1)
                )
                if ch2 == NTILE - 1:
                    scr3 = scrpool.tile([K, K], F32, tag="scr3")
                    nc.vector.scalar_tensor_tensor(
                        out=scr3[:],
                        in0=cps[:],
                        scalar=1.0,
                        in1=tr_f[:],
                        op0=MULT,
                        op1=MULT,
                        accum_out=stackA[:, 3 * BL + b : 3 * BL + b + 1],
                    )

            def edge_pairs_task():
                # start/end scores + the per-seq spurious-pair subtraction.
                oh0 = scrpool.tile([K, BL], BF16, tag="oh0")
                nc.vector.tensor_scalar(oh0[:], labs0[:], iotac[:], None, op0=EQ)
                nc.vector.tensor_scalar_mul(stackA[:, BL : 2 * BL], oh0[:], st_col[:])
                oh1 = scrpool.tile([K, BL], F32, tag="oh1")
                nc.vector.tensor_scalar(oh1[:], labs1[:], iotac[:], None, op0=EQ)
                nc.vector.tensor_scalar_mul(
                    stackA[:, 2 * BL : 3 * BL], oh1[:], en_col[:]
                )
                # U[m, p] = sum_i tr[i, m] * oh1[i, p] = tr[l_last[p], m]
                pu = pupool.tile([K, BL], F32, tag="pu")
                nc.tensor.matmul(pu[:], tr_f[:], oh1[:], start=True, stop=True)
                nc.vector.scalar_tensor_tensor(
                    out=stackA[:, 4 * BL : 5 * BL],
                    in0=pu[:],
                    scalar=-1.0,
                    in1=oh1[:],
                    op0=MULT,
                    op1=MULT,
                )

            # ---- prologue: fill XP[0] (and a bit more) ---------------------
            PRE = 48
            xq = 0
            for _ in range(PRE):
                xh_task(xq)
                xq += 1

            s0 = stpool.tile([K, 2 * BL], BF16, tag="st")
            nc.vector.tensor_scalar_mul(s0[:, 0:BL], XP[0][:, 0, 0:BL], stc0[:])
            nc.vector.tensor_scalar_mul(
                s0[:, BL : 2 * BL], XP[0][:, 0, BL : 2 * BL], een_col[:]
            )
            s_cur = s0

            # ---- chain + interleaved tasks ---------------------------------
            oq = 0
            eq = 0
            cq = 0
            done_edge = False
            for i in range(nsteps):
                r = 1 + i
                c, ti = divmod(r, CHUNK)
                ps = pspool.tile([K, 2 * BL], F32, tag="ps")
                nc.tensor.matmul(
                    ps[:, 0:BL], e_sb[:], s_cur[:, 0:BL], start=True, stop=True
                )
                nc.tensor.matmul(
                    ps[:, BL : 2 * BL], eT_sb[:], s_cur[:, BL : 2 * BL],
                    start=True, stop=True,
                )
                s_new = stpool.tile([K, 2 * BL], BF16, tag="st")
                nc.vector.tensor_mul(s_new[:], ps[:], XP[c][:, ti, :])
                s_cur = s_new

                if i % 2 == 0 and oq < NQ:
                    ohc_task(oq)
                    oq += 1
                if i % 4 == 0 and xq < NQ:
                    xh_task(xq)
                    xq += 1
                if (i % 4 == 2 or (i % 4 == 3 and i > 600)) and eq < NQ and eq < xq - 40:
                    emit_task(eq)
                    eq += 1
                if i >= 520 and i % 3 != 0 and cq < NQ:
                    count_task(cq)
                    cq += 1
                if i == 1000 and not done_edge:
                    edge_pairs_task()
                    done_edge = True

            while xq < NQ:
                xh_task(xq)
                xq += 1
            while oq < NQ:
                ohc_task(oq)
                oq += 1
            while eq < NQ:
                emit_task(eq)
                eq += 1
            while cq < NQ:
                count_task(cq)
                cq += 1
            if not done_edge:
                edge_pairs_task()

            # ---- combine: Z = p^T E w --------------------------------------
            qz = pspool.tile([K, 2 * BL], F32, tag="ps")
            nc.tensor.matmul(qz[:, 0:BL], e_sb[:], s_cur[:, 0:BL], start=True, stop=True)
            nc.vector.tensor_mul(stackA[:, 0:BL], qz[:, 0:BL], s_cur[:, BL : 2 * BL])

            # ---- partition sums via ones-matmuls ---------------------------
            fin = popool.tile([1, 1024], F32, tag="fin")
            nc.tensor.matmul(fin[:, 0:AW], ones96[:], stackA[:], start=True, stop=True)
            nc.tensor.matmul(
                fin[:, 512 : 512 + NQ], ones128[:], stackB[:], start=True, stop=True
            )
            nc.vector.tensor_copy(outstage[:, 0:AW], fin[:, 0:AW])
            nc.vector.tensor_copy(outstage[:, AW:], fin[:, 512 : 512 + NQ])
            nc.sync.dma_start(out=y_out[:], in_=outstage[:])

    nc.compile()
    return nc


_cached = {}


def _get_program():
    if "p" not in _cached:
        _cached["p"] = build_program()
    return _cached["p"]


def prep_in_maps(logits, labels, transitions, start_transitions, end_transitions):
    """Host-side prep: bf16 casts, half-split/reverse, label layouts."""
    lg = np.asarray(logits, np.float32)
    lab = np.asarray(labels).astype(np.int64)
    lgp = np.zeros((B, T, KP), np.float32)
    lgp[:, :, 0:K] = lg
    lgf = lgp[:, 0:HALF, :].astype(ml_dtypes.bfloat16)
    lgb = lgp[:, : HALF - 1 : -1, :].astype(ml_dtypes.bfloat16)
    labf = lab.astype(np.float32)
    labcur = np.concatenate([labf[:, 0:HALF], labf[:, : HALF - 1 : -1]], axis=1)
    labnext = np.concatenate(
        [labf[:, 1 : HALF + 1], labf[:, T - 1 : T], labf[:, : HALF : -1]], axis=1
    )
    lab_edge = np.stack([labf[:, 0], labf[:, T - 1]])
    tr = np.ascontiguousarray(transitions, np.float32)
    st = np.ascontiguousarray(start_transitions, np.float32).reshape(K, 1)
    en = np.ascontiguousarray(end_transitions, np.float32).reshape(K, 1)

    in_maps = []
    for cid in range(N_CORES):
        sl = slice(cid * BL, (cid + 1) * BL)
        in_maps.append(
            {
                "lgf": np.ascontiguousarray(lgf[sl]),
                "lgb": np.ascontiguousarray(lgb[sl]),
                "labcur": np.ascontiguousarray(labcur[sl]),
                "labnext": np.ascontiguousarray(labnext[sl]),
                "transitions": tr,
                "start_t": st,
                "end_t": en,
                "lab_edge": np.ascontiguousarray(lab_edge[:, sl]),
            }
        )
    return in_maps


def host_combine(y_rows):
    """Combine per-core output rows into the scalar loss."""
    total = 0.0
    for v in y_rows:
        v = np.asarray(v, np.float64).reshape(-1)
        zpre = v[0:BL]
        start_s = v[BL : 2 * BL]
        end_s = v[2 * BL : 3 * BL]
        trans_s = v[3 * BL : 4 * BL]
        negfix = v[4 * BL : 5 * BL]
        emit_s = v[AW:].reshape(NTILE, BL).sum(axis=0)
        logz = np.log(zpre) + (T - 1) * C0
        score = emit_s + trans_s + start_s + end_s + negfix
        total += (score - logz).sum()
    return np.float32(-total)


def kernel(logits, labels, mask, transitions, start_transitions, end_transitions):
    # mask is all-ones for this problem (spec fill=ones); it does not enter
    # the computation.
    nc = _get_program()
    in_maps = prep_in_maps(
        logits, labels, transitions, start_transitions, end_transitions
    )
    res = run_bass_kernel_spmd(nc, in_maps, core_ids=list(range(N_CORES)))
    return host_combine([res.results[c]["y"] for c in range(N_CORES)])


# revision 12
# speedup vs baseline: 2.1413x; 2.1413x over previous
"""CRF negative log-likelihood on 8 Trainium2 NeuronCores.

Problem: B=128, T=2048, K=96 linear-chain CRF loss (log-partition via the
forward algorithm minus the joint path score), mask is all-ones.

Strategy (v4)
-------------
Batch dim B is sharded 16 sequences per core (data parallel); the [K,K]
transition matrix is replicated.  The host splits the time axis in two:
lgf = logits[:, 0:1024] and lgb = logits[:, 2047:1023:-1] (back half
time-reversed), both bf16-cast.  With r the local index in either half,
the forward chain (p_r = xf_r * E^T p_{r-1}) and the backward chain
(w_r = xb_r * E w_{r-1}) share the same r sequencing and meet in the
middle: Z = p_1023^T E w_1023.

Per core, per chain step: TWO tiny PE matmuls ([96x96] @ [96,16], fwd
and bwd) into ONE PSUM tile [96,32], then ONE merged DVE multiply with
the paired xhat slice XP[c][:, :, ti] (a 32-element stride-128 read,
which the DVE handles at no extra cost).  Merging halves the DVE chain
instruction count vs per-direction multiplies, amortizing the fixed
~120-cycle PSUM access per DVE op.  xhat production keeps the proven
pipeline (DMA -> ACT exp -> PE transpose -> contiguous ACT copy into
the slot-major pair tile); the em DMAs ride the idle gpsimd SWDGE
queue in bf16, halving HBM traffic.

The joint score runs off the critical path: per (b, 128-step tile)
one-hot label tiles (DVE compare vs iota) give the emission score via a
fused multiply-reduce, and the transition score via PE pair-count
matmuls PSUM-accumulated over each sequence (16 matmuls -> ONE fused
<count, transitions> reduce per sequence).  The time-reversed back half
counts pairs in the same (i->j) orientation because the host supplies
labnext = label successors in (half, r) layout; the single spurious
pair per sequence is subtracted with a tiny one-hot/matmul batch.

Each core returns a small vector of partial sums; the host only
assembles the final scalar: loss = -sum_b (score_b - logZ_b).
"""
import sys

sys.path.insert(0, "/opt/trn_rl_repo")

import numpy as np
import ml_dtypes

import concourse.bacc as bacc
import concourse.mybir as mybir
from concourse.bass_utils import run_bass_kernel_spmd
from concourse.tile import TileContext

B, T, K = 128, 2048, 96
N_CORES = 8
BL = B // N_CORES          # 16 sequences per core
NSLOT = 2 * BL             # 32 pair slots (slot = b + 16*h)
C0 = 5.06                  # per-step scale offset, ~E[log growth]
CHUNK = 128                # time-steps per tile
HALF = T // 2              # 1024
NCH = HALF // CHUNK        # 8 chunks per half
NTILE = 2 * NCH            # 16 label tiles per sequence (ch2 = 2c + h)
NQ = NTILE * BL            # 256 quanta
F32 = mybir.dt.float32
BF16 = mybir.dt.bfloat16
I32 = mybir.dt.int32
EXP = mybir.ActivationFunctionType.Exp
COPY = mybir.ActivationFunctionType.Copy
MULT = mybir.AluOpType.mult
EQ = mybir.AluOpType.is_equal

# stackA columns: [0:16] zpre, [16:32] start, [32:48] end, [48:64] trans,
# [64:80] negative pair corrections.  stackB: [128, 256] emission partials.
AW = 5 * BL
OUT_W = AW + NQ


def build_program():
    nsteps = HALF - 1                  # 1023 chain steps (r = 1..1023)

    nc = bacc.Bacc(None, target_bir_lowering=False)
    lgf_in = nc.declare_dram_parameter("lgf", [BL, HALF, K], BF16, isOutput=False)
    lgb_in = nc.declare_dram_parameter("lgb", [BL, HALF, K], BF16, isOutput=False)
    labc_in = nc.declare_dram_parameter("labcur", [BL, T], F32, isOutput=False)
    labn_in = nc.declare_dram_parameter("labnext", [BL, T], F32, isOutput=False)
    tr_in = nc.declare_dram_parameter("transitions", [K, K], F32, isOutput=False)
    st_in = nc.declare_dram_parameter("start_t", [K, 1], F32, isOutput=False)
    en_in = nc.declare_dram_parameter("end_t", [K, 1], F32, isOutput=False)
    le_in = nc.declare_dram_parameter("lab_edge", [2, BL], F32, isOutput=False)
    y_out = nc.declare_dram_parameter("y", [1, OUT_W], F32, isOutput=True)

    # (c, h) order for ch2 so XP[c] completes as early as possible
    CH2 = [(c, h) for c in range(NCH) for h in range(2)]

    with TileContext(nc) as tc:
        with (
            tc.tile_pool(name="const", bufs=1) as cpool,
            tc.tile_pool(name="xp", bufs=1) as xpool,
            tc.tile_pool(name="ohc", bufs=1) as ohpool,
            tc.tile_pool(name="em", bufs=1) as empool,
            tc.tile_pool(name="state", bufs=4) as stpool,
            tc.tile_pool(name="scr", bufs=4) as scrpool,
            tc.tile_pool(name="stacks", bufs=1) as kpool,
            tc.tile_pool(name="ps", bufs=2, space="PSUM") as pspool,
            tc.tile_pool(name="psx", bufs=2, space="PSUM") as psxpool,
            tc.tile_pool(name="cps", bufs=2, space="PSUM") as cpspool,
            tc.tile_pool(name="po", bufs=1, space="PSUM") as popool,
        ):
            # ---- constants -------------------------------------------------
            tr_f = cpool.tile([K, K], F32, tag="tr_f")
            trT_f = cpool.tile([K, K], F32, tag="trT_f")
            nc.sync.dma_start(out=tr_f[:], in_=tr_in[:])
            nc.sync.dma_start(out=trT_f[:], in_=tr_in[:].rearrange("i j -> j i"))
            e_sb = cpool.tile([K, K], BF16, tag="e_sb")
            eT_sb = cpool.tile([K, K], BF16, tag="eT_sb")
            nc.scalar.activation(e_sb[:], tr_f[:], EXP)
            nc.scalar.activation(eT_sb[:], trT_f[:], EXP)

            st_col = cpool.tile([K, 1], F32, tag="st_col")
            en_col = cpool.tile([K, 1], F32, tag="en_col")
            nc.sync.dma_start(out=st_col[:], in_=st_in[:])
            nc.sync.dma_start(out=en_col[:], in_=en_in[:])
            een_col = cpool.tile([K, 1], F32, tag="een_col")
            nc.scalar.activation(een_col[:], en_col[:], EXP)
            labs0 = cpool.tile([K, BL], F32, tag="labs0")
            labs1 = cpool.tile([K, BL], F32, tag="labs1")
            nc.sync.dma_start(out=labs0[:], in_=le_in[0:1, :].to_broadcast([K, BL]))
            nc.sync.dma_start(out=labs1[:], in_=le_in[1:2, :].to_broadcast([K, BL]))
            iotac_i = cpool.tile([K, 1], I32, tag="iotac_i")
            nc.gpsimd.iota(iotac_i[:], pattern=[[1, 1]], base=0, channel_multiplier=1)
            iotac = cpool.tile([K, 1], F32, tag="iotac")
            nc.vector.tensor_copy(iotac[:], iotac_i[:])

            negc0 = cpool.tile([CHUNK, 1], F32, tag="negc0")
            nc.vector.memset(negc0[:], -C0)
            posc0 = cpool.tile([K, 1], F32, tag="posc0")
            nc.vector.memset(posc0[:], C0)
            stc0 = cpool.tile([K, 1], F32, tag="stc0")
            nc.scalar.activation(stc0[:], st_col[:], EXP, bias=posc0[:])
            # identity for PE transposes
            ones2d = cpool.tile([CHUNK, CHUNK], BF16, tag="ones2d")
            nc.vector.memset(ones2d[:], 1.0)
            ident = cpool.tile([CHUNK, CHUNK], BF16, tag="ident")
            nc.gpsimd.affine_select(
                ident[:], ones2d[:], pattern=[[1, CHUNK]],
                compare_op=EQ, fill=0.0, base=0, channel_multiplier=-1,
            )
            iota_i = cpool.tile([CHUNK, K], I32, tag="iota_i")
            nc.gpsimd.iota(iota_i[:], pattern=[[1, K]], base=0, channel_multiplier=0)
            iota = cpool.tile([CHUNK, K], BF16, tag="iota")
            nc.vector.tensor_copy(iota[:], iota_i[:])
            ones96 = cpool.tile([K, 1], F32, tag="ones96")
            ones128 = cpool.tile([CHUNK, 1], F32, tag="ones128")
            nc.vector.memset(ones96[:], 1.0)
            nc.vector.memset(ones128[:], 1.0)

            lab_sb = []
            labn_sb = []
            for b in range(BL):
                lt = cpool.tile([CHUNK, NTILE], F32, tag=f"lab{b}")
                nc.sync.dma_start(
                    out=lt[:],
                    in_=labc_in[b : b + 1, :].rearrange("o (c t) -> (o t) c", t=CHUNK),
                )
                lab_sb.append(lt)
                ln = cpool.tile([CHUNK, NTILE], F32, tag=f"labn{b}")
                nc.sync.dma_start(
                    out=ln[:],
                    in_=labn_in[b : b + 1, :].rearrange("o (c t) -> (o t) c", t=CHUNK),
                )
                labn_sb.append(ln)

            stackA = kpool.tile([K, AW], F32, tag="stackA")
            stackB = kpool.tile([CHUNK, NQ], F32, tag="stackB")
            outstage = kpool.tile([1, OUT_W], F32, tag="outstage")
            nc.vector.memset(stackA[:], 0.0)

            # slot-major xhat pairs: XP[c][:, slot, ti] = xhat at r=c*128+ti
            # for slot = b (fwd half) or 16+b (reversed half)
            XP = [
                xpool.tile([K, NSLOT, CHUNK], BF16, tag=f"xp{c}", name=f"xp{c}")
                for c in range(NCH)
            ]

            em_tiles = {}
            ohc_tiles = {}

            def xh_task(q):
                ch2, b = divmod(q, BL)
                c, h = CH2[ch2]
                src = lgf_in if h == 0 else lgb_in
                em = empool.tile(
                    [CHUNK, K], BF16, tag=f"em{q % 96}", name=f"em{q}"
                )
                nc.gpsimd.dma_start(
                    out=em[:], in_=src[b, c * CHUNK : (c + 1) * CHUNK, :]
                )
                em_tiles[q] = em
                etile = scrpool.tile([CHUNK, K], BF16, tag="etile")
                nc.scalar.activation(etile[:], em[:], EXP, bias=negc0[:])
                psx = psxpool.tile([K, CHUNK], BF16, tag="psx")
                nc.tensor.transpose(psx[:], etile[:], ident[:])
                nc.scalar.activation(XP[c][:, b + BL * h, :], psx[:], COPY)

            def ohc_task(o):
                ch2, b = divmod(o, BL)
                oh = ohpool.tile([CHUNK, K], BF16, tag=f"ohc{o}", name=f"ohc{o}")
                nc.vector.tensor_scalar(
                    oh[:], iota[:], lab_sb[b][:, ch2 : ch2 + 1], None, op0=EQ
                )
                ohc_tiles[o] = oh

            def emit_task(s):
                em = em_tiles.pop(s)
                scr = scrpool.tile([CHUNK, K], BF16, tag="scr")
                nc.vector.scalar_tensor_tensor(
                    out=scr[:],
                    in0=ohc_tiles[s][:],
                    scalar=1.0,
                    in1=em[:],
                    op0=MULT,
                    op1=MULT,
                    accum_out=stackB[:, s : s + 1],
                )

            cps_cur = [None]

            def count_task(n):
                b, ch2 = divmod(n, NTILE)
                o = ch2 * BL + b
                oh = ohc_tiles[o]
                ohn = scrpool.tile([CHUNK, K], BF16, tag=f"ohn{n % 4}")
                nc.vector.tensor_scalar(
                    ohn[:], iota[:], labn_sb[b][:, ch2 : ch2 + 1], None, op0=EQ
                )
                if ch2 == 0:
                    cps_cur[0] = cpspool.tile([K, K], F32, tag="cps", name="cps")
                cps = cps_cur[0]
                nc.tensor.matmul(
                    cps[:], oh[:], ohn[:], start=(ch2 == 0), stop=(ch2 == NTILE - 1)
                )
                if ch2 == NTILE - 1:
                    scr3 = scrpool.tile([K, K], F32, tag="scr3")
                    nc.vector.scalar_tensor_tensor(
                        out=scr3[:],
                        in0=cps[:],
                        scalar=1.0,
                        in1=tr_f[:],
                        op0=MULT,
                        op1=MULT,
                        accum_out=stackA[:, 3 * BL + b : 3 * BL + b + 1],
                    )

            def edge_pairs_task():
                # start/end scores + the per-seq spurious-pair subtraction
                # (labnext[r=0] of the reversed half pairs (l_{T-1}, l_{T-1}))
                oh0 = scrpool.tile([K, BL], BF16, tag="oh0")
                nc.vector.tensor_scalar(oh0[:], labs0[:], iotac[:], None, op0=EQ)
                nc.vector.tensor_scalar_mul(stackA[:, BL : 2 * BL], oh0[:], st_col[:])
                oh1 = scrpool.tile([K, BL], F32, tag="oh1")
                nc.vector.tensor_scalar(oh1[:], labs1[:], iotac[:], None, op0=EQ)
                nc.vector.tensor_scalar_mul(
                    stackA[:, 2 * BL : 3 * BL], oh1[:], en_col[:]
                )
                # U[m, p] = sum_i tr[i, m] * oh1[i, p] = tr[l_last[p], m]
                pu = pspool.tile([K, NSLOT], F32, tag="ps", name="pu")
                nc.tensor.matmul(pu[:, 0:BL], tr_f[:], oh1[:], start=True, stop=True)
                nc.vector.scalar_tensor_tensor(
                    out=stackA[:, 4 * BL : 5 * BL],
                    in0=pu[:, 0:BL],
                    scalar=-1.0,
                    in1=oh1[:],
                    op0=MULT,
                    op1=MULT,
                )

            # ---- prologue: fill XP[0] (and a bit more) ---------------------
            PRE = 48
            xq = 0
            for _ in range(PRE):
                xh_task(xq)
                xq += 1

            s0 = stpool.tile([K, NSLOT], BF16, tag="st")
            nc.vector.tensor_scalar_mul(s0[:, 0:BL], XP[0][:, 0:BL, 0], stc0[:])
            nc.vector.tensor_scalar_mul(
                s0[:, BL:NSLOT], XP[0][:, BL:NSLOT, 0], een_col[:]
            )
            s_cur = s0

            # ---- chain + interleaved tasks ---------------------------------
            oq = 0
            eq = 0
            cq = 0
            done_edge = False
            for i in range(nsteps):
                r = 1 + i
                c, ti = divmod(r, CHUNK)
                ps = pspool.tile([K, NSLOT], F32, tag="ps")
                nc.tensor.matmul(
                    ps[:, 0:BL], e_sb[:], s_cur[:, 0:BL], start=True, stop=True
                )
                nc.tensor.matmul(
                    ps[:, BL:NSLOT], eT_sb[:], s_cur[:, BL:NSLOT],
                    start=True, stop=True,
                )
                s_new = stpool.tile([K, NSLOT], BF16, tag="st")
                nc.vector.tensor_mul(s_new[:], ps[:], XP[c][:, :, ti])
                s_cur = s_new

                if i % 2 == 0 and oq < NQ:
                    ohc_task(oq)
                    oq += 1
                if i % 4 == 0 and xq < NQ:
                    xh_task(xq)
                    xq += 1
                if (i % 4 == 2 or (i % 4 == 3 and i > 600)) and eq < NQ and eq < xq - 40:
                    emit_task(eq)
                    eq += 1
                if i >= 500 and i % 2 == 1 and cq < NQ:
                    count_task(cq)
                    cq += 1
                if i == 990 and not done_edge:
                    edge_pairs_task()
                    done_edge = True

            while xq < NQ:
                xh_task(xq)
                xq += 1
            while oq < NQ:
                ohc_task(oq)
                oq += 1
            while eq < NQ:
                emit_task(eq)
                eq += 1
            while cq < NQ:
                count_task(cq)
                cq += 1
            if not done_edge:
                edge_pairs_task()

            # ---- combine: Z = p^T E w --------------------------------------
            qz = pspool.tile([K, NSLOT], F32, tag="ps")
            nc.tensor.matmul(qz[:, 0:BL], e_sb[:], s_cur[:, 0:BL], start=True, stop=True)
            nc.vector.tensor_mul(stackA[:, 0:BL], qz[:, 0:BL], s_cur[:, BL:NSLOT])

            # ---- partition sums via ones-matmuls ---------------------------
            fin = popool.tile([1, 1024], F32, tag="fin")
            nc.tensor.matmul(fin[:, 0:AW], ones96[:], stackA[:], start=True, stop=True)
            nc.tensor.matmul(
                fin[:, 512 : 512 + NQ], ones128[:], stackB[:], start=True, stop=True
            )
            nc.vector.tensor_copy(outstage[:, 0:AW], fin[:, 0:AW])
            nc.vector.tensor_copy(outstage[:, AW:], fin[:, 512 : 512 + NQ])
            nc.sync.dma_start(out=y_out[:], in_=outstage[:])

    nc.compile()
    return nc


_cached = {}


def _get_program():
    if "p" not in _cached:
        _cached["p"] = build_program()
    return _cached["p"]


def prep_in_maps(logits, labels, transitions, start_transitions, end_transitions):
    """Host-side prep: bf16 casts, half-split/reverse, label layouts."""
    lg = np.asarray(logits, np.float32)
    lab = np.asarray(labels).astype(np.int64)
    lgf = lg[:, 0:HALF, :].astype(ml_dtypes.bfloat16)
    lgb = lg[:, : HALF - 1 : -1, :].astype(ml_dtypes.bfloat16)
    labf = lab.astype(np.float32)
    labcur = np.concatenate([labf[:, 0:HALF], labf[:, : HALF - 1 : -1]], axis=1)
    labnext = np.concatenate(
        [labf[:, 1 : HALF + 1], labf[:, T - 1 : T], labf[:, : HALF : -1]], axis=1
    )
    lab_edge = np.stack([labf[:, 0], labf[:, T - 1]])
    tr = np.ascontiguousarray(transitions, np.float32)
    st = np.ascontiguousarray(start_transitions, np.float32).reshape(K, 1)
    en = np.ascontiguousarray(end_transitions, np.float32).reshape(K, 1)

    in_maps = []
    for cid in range(N_CORES):
        sl = slice(cid * BL, (cid + 1) * BL)
        in_maps.append(
            {
                "lgf": np.ascontiguousarray(lgf[sl]),
                "lgb": np.ascontiguousarray(lgb[sl]),
                "labcur": np.ascontiguousarray(labcur[sl]),
                "labnext": np.ascontiguousarray(labnext[sl]),
                "transitions": tr,
                "start_t": st,
                "end_t": en,
                "lab_edge": np.ascontiguousarray(lab_edge[:, sl]),
            }
        )
    return in_maps


def host_combine(y_rows):
    """Combine per-core output rows into the scalar loss."""
    total = 0.0
    for v in y_rows:
        v = np.asarray(v, np.float64).reshape(-1)
        zpre = v[0:BL]
        start_s = v[BL : 2 * BL]
        end_s = v[2 * BL : 3 * BL]
        trans_s = v[3 * BL : 4 * BL]
        negfix = v[4 * BL : 5 * BL]
        emit_s = v[AW:].reshape(NTILE, BL).sum(axis=0)
        logz = np.log(zpre) + (T - 1) * C0
        score = emit_s + trans_s + start_s + end_s + negfix
        total += (score - logz).sum()
    return np.float32(-total)


def kernel(logits, labels, mask, transitions, start_transitions, end_transitions):
    # mask is all-ones for this problem (spec fill=ones); it does not enter
    # the computation.
    nc = _get_program()
    in_maps = prep_in_maps(
        logits, labels, transitions, start_transitions, end_transitions
    )
    res = run_bass_kernel_spmd(nc, in_maps, core_ids=list(range(N_CORES)))
    return host_combine([res.results[c]["y"] for c in range(N_CORES)])
